# revision 23
# baseline (speedup 1.0000x reference)
"""AdaptiveThresholdLIFNeuron Trainium2 kernel (8 NeuronCores, SPMD).

The per-step global spike-rate EMA couples every element each timestep; on
this toolchain any cross-core exchange costs >=5us/step (collective floor;
the SWDGE remote-DMA ucode is absent from the runtime image), i.e. >=1.3ms
for T=256 just in communication. So the elementwise recurrence is
REPLICATED: every core runs the identical full-width [128 x 1024] chain
(bit-identical EMA evolution, zero cross-core traffic) and each core writes
only its own 1/8 of the spike output.

Host prep is layout-only: x [B,F,T] -> time-major XT[t][g][w] so the device
streams 512KB/step contiguously; each core's input is column-rotated so its
output shard sits at columns [0:128).

Engine schedule per step t (f32, W=1024 columns; DVE owns cols [0:DCOL),
Pool owns [DCOL:W) for the four split ops):
  Act:  TH = K*A + C ; THa = alpha*TH (pool half) ; u8 out of SPK shard ;
        r1 = relu(E - 0.01) ; C = 0.1*r1 + base ; SYb = beta*SY (pool
        half) ; Ma = alpha*M (pool half)
  DVE:  SPK = (M >= TH) [accum -> S_t] ; R.d = (alpha*SPK)*TH ;
        E = 0.99E + Spsum ; M.d = M0 - R ; A = gamma*A + M [accum] ;
        SY.d = beta*SY + X[t+2] ; M0.d = alpha*M + SY
  Pool: R.p = SPK*THa ; M.p = M0 - R ; SY.p = SYb + X[t+2] ;
        M0.p = Ma + SY
  PE:   Spsum = (c*ones) @ sums[:, t]     (partition reduce + broadcast)

M0 is software-pipelined one step ahead so the Act TH op overlaps the
SY/M0 updates. mem/th traces, ema and adapt-mean are finished on the host
from exact per-step sums.
"""

import numpy as np

B, F, T = 32, 4096, 256
N = B * F
G = 128
W = N // G                    # 1024
NCORES = 8
SHARD = W // NCORES           # 128
DCOL = 864                    # DVE columns of split ops; Pool gets W-DCOL

THRESHOLD_BASE = 1.0
DT = 0.001
ALPHA = float(np.exp(-DT / 0.02))
BETA = float(np.exp(-DT / 0.005))
GAMMA = float(np.exp(-DT / 0.1))
EMA_DECAY = 0.99
ADAPT_STRENGTH = 0.1
HOMEO_RATE = 0.01
K = float(np.float32(0.1) * (np.float32(1.0) - np.float32(GAMMA)))
C1 = float(np.float32(0.01) / np.float32(N))

BK = 8
NBLK = T // BK

_CACHE = {}

# ------------------------------------------------------------ V ledger
# V prologue (14 ops): 1 aux-memset, 2 SY-ms, 3 M-ms, 4 A-ms, 5 E-ms,
# 6 R-ms, 7 cones-ms, 8 negHR-ms, 9 base, 10 Ccopy, 11 SY0 (syn_0),
# 12 Mp (m_0), 13 A0 (asums[0]), 14 SYup (syn_1).  (M0 init is on PE.)
# Per step t<=253 (5): SPK, R.d, MFIX(full), A, SY.d
# t=254 (4): SPK, R.d, MFIX, A ; t=255 (3): SPK, Rfin, MPL
PRE = 14


def _vbase(t):
    return PRE + 5 * t


def _idx_spk(t):
    if t == T - 1:
        return _vbase(T - 2) + 4 + 1
    return _vbase(t) + 1


def _idx_A(t):
    # V op producing "A entering step t"
    return 13 if t == 0 else _vbase(t - 1) + 4


def _idx_mfix(t):
    return _vbase(t) + 3


def _idx_syd(t):
    # V SY.d update issued during step t (produces syn_{t+2})
    return _vbase(t) + 5


def _final_dve():
    return _vbase(T - 2) + 4 + 3


# ---------------------------------------------------------- Pool ledger
# per step t<=253 (4): R.p, MFIX.p, SY.p, M0.p ; t=254 (2): R.p, MFIX.p
def _p_rp(t):
    return 2 * t + 1


def _p_sy(t):
    return 2 * t + 2


def _final_pool():
    return 2 * (T - 2) + 1


# ----------------------------------------------------------- Act ledger
# per step t<=253 (7): TH, THa, u8, Ecopy, r1, C, SYb
# t=254 (6): TH, THa, u8, Ecopy, r1, C ; t=255 (2): TH, u8
def _a_th(t):
    if t == T - 1:
        return 7 * (T - 2) + 6 + 1
    return 7 * t + 1


def _a_tha(t):
    return 7 * t + 2


def _a_r1(t):
    return 7 * t + 5


def _a_syb(t):
    return 7 * t + 7


def _final_act():
    return _a_th(T - 1) + 1


def _build():
    import concourse.bass as bass
    import concourse.bacc as bacc
    import concourse.mybir as mybir

    f32 = mybir.dt.float32
    u8 = mybir.dt.uint8
    AOP = mybir.AluOpType
    ACT = mybir.ActivationFunctionType

    nc = bacc.Bacc(None, target_bir_lowering=False, debug=False)

    xt = nc.declare_dram_parameter("xt", [T * G, W], f32, isOutput=False)
    cfg = nc.declare_dram_parameter("cfg", [G, 2], f32, isOutput=False)
    eye = nc.declare_dram_parameter("eye", [G, G], f32, isOutput=False)
    eyea_in = nc.declare_dram_parameter("eyea", [G, G], f32, isOutput=False)
    eye1_in = nc.declare_dram_parameter("eye1", [G, G], f32, isOutput=False)
    spk_out = nc.declare_dram_parameter("spk", [G, SHARD * T], u8, isOutput=True)
    AUXW = 4 * T + 2
    aux_out = nc.declare_dram_parameter("aux", [G, AUXW], f32, isOutput=True)

    PCOL = W - DCOL

    from contextlib import ExitStack
    es = ExitStack()
    with es:
        sb = lambda name, shape, dt_: es.enter_context(
            nc.sbuf_tensor(name, shape, dt_))
        xbuf = sb("xbuf", [G, 2 * BK * W], f32)
        SY = sb("SY", [G, W], f32)
        M = sb("M", [G, W], f32)
        A_ = sb("A_", [G, W], f32)
        TH = sb("TH", [G, W], f32)
        SPK = sb("SPK", [G, W], f32)
        R = sb("R", [G, W], f32)
        M0 = sb("M0", [G, W], f32)
        THA = sb("THA", [G, PCOL], f32)
        SYB = sb("SYB", [G, PCOL], f32)
        MA = sb("MA", [G, PCOL], f32)
        u8acc = sb("u8acc", [G, SHARD * T], u8)
        aux = sb("auxb", [G, AUXW], f32)
        E = sb("E", [G, 1], f32)
        baset = sb("base", [G, 1], f32)
        C = sb("C", [G, 1], f32)
        r1 = sb("r1", [G, 1], f32)
        negHR = sb("negHR", [G, 1], f32)
        cfgb = sb("cfgb", [G, 2], f32)
        cones = sb("cones", [G, G], f32)
        eye99 = sb("eye99", [G, G], f32)
        eyea = sb("eyea_b", [G, G], f32)
        eye1 = sb("eye1_b", [G, G], f32)
        Spsum = es.enter_context(nc.psum_tensor("Spsum", [G, 1], f32))
        M0ps = es.enter_context(nc.psum_tensor("M0ps", [G, W], f32))
        dma_in = es.enter_context(nc.semaphore("dma_in"))
        cfg_sem = es.enter_context(nc.semaphore("cfg_sem"))
        dsem = es.enter_context(nc.semaphore("dsem"))
        asem = es.enter_context(nc.semaphore("asem"))
        poolc = es.enter_context(nc.semaphore("poolc"))
        psem = es.enter_context(nc.semaphore("psem"))
        osem = es.enter_context(nc.semaphore("osem"))
        block = es.enter_context(nc.Block())

        def xslice(t):
            s = (t % (2 * BK)) * W
            return xbuf[:, s:s + W]

        def xslice_d(t):
            s = (t % (2 * BK)) * W
            return xbuf[:, s:s + DCOL]

        def xslice_p(t):
            s = (t % (2 * BK)) * W + DCOL
            return xbuf[:, s:s + PCOL]

        dd = slice(0, DCOL)
        pp = slice(DCOL, W)

        @block.sync
        def _(sync):
            sync.dma_start(out=cfgb[:, :], in_=cfg[:, :]).then_inc(cfg_sem, 16)
            sync.dma_start(out=eye99[:, :], in_=eye[:, :]).then_inc(cfg_sem, 16)
            sync.dma_start(out=eyea[:, :], in_=eyea_in[:, :]).then_inc(cfg_sem, 16)
            sync.dma_start(out=eye1[:, :], in_=eye1_in[:, :]).then_inc(cfg_sem, 16)
            for b in range(NBLK):
                if b >= 2:
                    tcons = (b - 1) * BK - 3
                    sync.wait_ge(dsem, _idx_syd(tcons))
                    sync.wait_ge(poolc, _p_sy(tcons))
                sync.dma_start(
                    out=xbuf[:, ((b % 2) * BK * W):((b % 2) * BK * W + BK * W)]
                    .rearrange("g (t w) -> g t w", t=BK),
                    in_=xt[b * BK * G:(b + 1) * BK * G, :].rearrange(
                        "(t g) w -> g t w", g=G
                    ),
                ).then_inc(dma_in, 16)

        @block.vector
        def _(vector):
            cnt = [0]

            def V(ins):
                cnt[0] += 1
                return ins.then_inc(dsem, 1)

            def vw(sem, val):
                if val > 0:
                    vector.wait_ge(sem, val)

            def selfw():
                vw(dsem, cnt[0])

            # prologue 1..14
            V(vector.memset(aux[:, :], 0.0))
            V(vector.memset(SY[:, :], 0.0))
            V(vector.memset(M[:, :], 0.0))
            V(vector.memset(A_[:, :], 0.0))
            V(vector.memset(E[:, :], 0.0))
            V(vector.memset(R[:, :], 0.0))
            V(vector.memset(cones[:, :], C1))
            V(vector.memset(negHR[:, :], -float(HOMEO_RATE)))
            vw(cfg_sem, 64)
            V(vector.tensor_tensor(out=baset[:, :], in0=cfgb[:, 0:1],
                                   in1=cfgb[:, 1:2], op=AOP.add))
            selfw()
            V(vector.tensor_copy(out=C[:, :], in_=baset[:, :]))
            vw(dma_in, 16)
            selfw()
            V(vector.scalar_tensor_tensor(
                out=SY[:, :], in0=SY[:, :], scalar=BETA, in1=xslice(0),
                op0=AOP.mult, op1=AOP.add))
            selfw()
            V(vector.scalar_tensor_tensor(
                out=M[:, :], in0=M[:, :], scalar=ALPHA, in1=SY[:, :],
                op0=AOP.mult, op1=AOP.add))
            selfw()
            V(vector.scalar_tensor_tensor(
                out=A_[:, :], in0=A_[:, :], scalar=GAMMA, in1=M[:, :],
                op0=AOP.mult, op1=AOP.add,
                accum_out=aux[:, 2 * T + 1:2 * T + 2]))
            selfw()
            V(vector.scalar_tensor_tensor(
                out=SY[:, :], in0=SY[:, :], scalar=BETA, in1=xslice(1),
                op0=AOP.mult, op1=AOP.add))

            for t in range(T):
                # SPK (full width, accum -> S_t)
                vw(asem, _a_th(t))
                if t >= 1:
                    vw(poolc, _p_rp(t - 1))
                selfw()
                V(vector.scalar_tensor_tensor(
                    out=SPK[:, :], in0=M[:, :], scalar=0.0, in1=TH[:, :],
                    op0=AOP.bypass, op1=AOP.is_ge,
                    accum_out=aux[:, t:t + 1]))
                if t == T - 1:
                    selfw()
                    V(vector.scalar_tensor_tensor(
                        out=R[:, :], in0=SPK[:, :], scalar=1.0, in1=TH[:, :],
                        op0=AOP.mult, op1=AOP.mult))
                    selfw()
                    V(vector.scalar_tensor_tensor(
                        out=M0[:, :], in0=R[:, :], scalar=-1.0, in1=M[:, :],
                        op0=AOP.mult, op1=AOP.add,
                        accum_out=aux[:, 2 * T:2 * T + 1]))
                    break
                # R.d
                selfw()
                V(vector.scalar_tensor_tensor(
                    out=R[:, dd], in0=SPK[:, dd], scalar=ALPHA, in1=TH[:, dd],
                    op0=AOP.mult, op1=AOP.mult))
                # MFIX (full width; R pool half + M0 from PE psum)
                vw(poolc, _p_rp(t))
                vw(psem, 6 * t + 4)
                selfw()
                V(vector.scalar_tensor_tensor(
                    out=M[:, :], in0=R[:, :], scalar=-1.0, in1=M0ps[:, :],
                    op0=AOP.mult, op1=AOP.add))
                # A (full width)
                selfw()
                V(vector.scalar_tensor_tensor(
                    out=A_[:, :], in0=A_[:, :], scalar=GAMMA, in1=M[:, :],
                    op0=AOP.mult, op1=AOP.add,
                    accum_out=aux[:, 2 * T + 1 + t + 1:2 * T + 2 + t + 1]))
                if t <= T - 3:
                    # SY.d (syn_{t+2}); PE must have read SY for M0-pair t
                    vw(dma_in, 16 * ((t + 2) // BK + 1))
                    vw(psem, 6 * t + 4)
                    selfw()
                    V(vector.scalar_tensor_tensor(
                        out=SY[:, dd], in0=SY[:, dd], scalar=BETA,
                        in1=xslice_d(t + 2), op0=AOP.mult, op1=AOP.add))

        @block.scalar
        def _(scalar):
            cnt = [0]

            def S(ins):
                cnt[0] += 1
                return ins.then_inc(asem, 1)

            def sw(sem, val):
                if val > 0:
                    scalar.wait_ge(sem, val)

            for t in range(T):
                # TH
                sw(dsem, max(_idx_A(t), 10))
                if t >= 1:
                    sw(poolc, _p_rp(t - 1))
                sw(asem, cnt[0])
                S(scalar.activation(
                    out=TH[:, :], in_=A_[:, :], func=ACT.Identity,
                    scale=K, bias=C[:, :]))
                if t == T - 1:
                    sw(dsem, _idx_spk(t))
                    sw(asem, cnt[0])
                    S(scalar.activation(
                        out=u8acc[:, t:t + (SHARD - 1) * T + 1:T],
                        in_=SPK[:, 0:SHARD], func=ACT.Copy))
                    break
                # THa (pool half)
                sw(asem, cnt[0])
                S(scalar.activation(
                    out=THA[:, :], in_=TH[:, pp], func=ACT.Identity,
                    scale=ALPHA))
                # u8
                sw(dsem, _idx_spk(t))
                sw(asem, cnt[0])
                S(scalar.activation(
                    out=u8acc[:, t:t + (SHARD - 1) * T + 1:T],
                    in_=SPK[:, 0:SHARD], func=ACT.Copy))
                # Ecopy: E <- Epsum (E' from PE)
                sw(psem, 6 * t + 6)
                sw(asem, cnt[0])
                S(scalar.activation(
                    out=E[:, :], in_=Spsum[:, :], func=ACT.Copy))
                # r1 = relu(E' - 0.01)  (read from psum)
                sw(asem, cnt[0])
                S(scalar.activation(
                    out=r1[:, :], in_=Spsum[:, :], func=ACT.Relu,
                    scale=1.0, bias=negHR[:, :]))
                # C = 0.1*r1 + base
                sw(asem, cnt[0])
                S(scalar.activation(
                    out=C[:, :], in_=r1[:, :], func=ACT.Identity,
                    scale=float(ADAPT_STRENGTH), bias=baset[:, :]))
                if t <= T - 3:
                    # SYb (pool half)
                    if t >= 1:
                        sw(poolc, _p_sy(t - 1))
                    else:
                        sw(dsem, 14)
                    sw(asem, cnt[0])
                    S(scalar.activation(
                        out=SYB[:, :], in_=SY[:, pp], func=ACT.Identity,
                        scale=BETA))

        @block.tensor
        def _(tensor):
            tensor.wait_ge(cfg_sem, 64)
            tensor.wait_ge(dsem, 14)  # memsets + prologue (M=m_0, SY=syn_1)
            for h in (slice(0, 512), slice(512, W)):
                tensor.matmul(
                    M0ps[:, h], eyea[:, :], M[:, h],
                    start=True, stop=False,
                ).then_inc(psem, 1)
                tensor.matmul(
                    M0ps[:, h], eye1[:, :], SY[:, h],
                    start=False, stop=True,
                ).then_inc(psem, 1)
            for t in range(T):
                tensor.wait_ge(dsem, _idx_spk(t))
                if t > 0:
                    tensor.wait_ge(asem, _a_r1(t - 1))
                tensor.matmul(
                    Spsum[:, :], eye99[:, :], E[:, :],
                    start=True, stop=False,
                ).then_inc(psem, 1)
                tensor.matmul(
                    Spsum[:, :], cones[:, :], aux[:, t:t + 1],
                    start=False, stop=True,
                ).then_inc(psem, 1)
                if t <= T - 3:
                    # M0 pair for step t+1: needs MFIX_t + syn_{t+2}
                    tensor.wait_ge(dsem, _idx_syd(t))
                    tensor.wait_ge(poolc, _p_sy(t))
                    for h in (slice(0, 512), slice(512, W)):
                        tensor.matmul(
                            M0ps[:, h], eyea[:, :], M[:, h],
                            start=True, stop=False,
                        ).then_inc(psem, 1)
                        tensor.matmul(
                            M0ps[:, h], eye1[:, :], SY[:, h],
                            start=False, stop=True,
                        ).then_inc(psem, 1)

        @block.gpsimd
        def _(pool):
            cnt = [0]

            def P(ins):
                cnt[0] += 1
                return ins.then_inc(poolc, 1)

            def pw(sem, val):
                if val > 0:
                    pool.wait_ge(sem, val)

            for t in range(T - 1):
                # R.p = SPK * THA
                pw(dsem, _idx_spk(t))
                pw(asem, _a_tha(t))
                pw(poolc, cnt[0])
                P(pool.tensor_tensor(out=R[:, pp], in0=SPK[:, pp],
                                     in1=THA[:, :], op=AOP.mult))
                if t <= T - 3:
                    # SY.p = SYB + X[t+2]; PE must have read SY (M0 pair t)
                    pw(asem, _a_syb(t))
                    pw(dma_in, 16 * ((t + 2) // BK + 1))
                    pw(psem, 6 * t + 4)
                    pw(poolc, cnt[0])
                    P(pool.tensor_tensor(out=SY[:, pp], in0=SYB[:, :],
                                         in1=xslice_p(t + 2), op=AOP.add))

            pool.wait_ge(dsem, _final_dve())
            pool.wait_ge(asem, _final_act())
            pool.wait_ge(poolc, _final_pool())
            pool.dma_start(out=spk_out[:, :], in_=u8acc[:, :]).then_inc(osem, 16)
            pool.dma_start(out=aux_out[:, :], in_=aux[:, :]).then_inc(osem, 16)
            pool.wait_ge(osem, 32)

    nc.finalize()
    return nc


def _prep_inputs(input_current, threshold_scale, adaptation_bias):
    x = np.ascontiguousarray(
        np.asarray(input_current, np.float32).reshape(N, T))
    XT = np.ascontiguousarray(x.reshape(G, W, T).transpose(2, 0, 1))
    cfg = np.zeros((G, 2), np.float32)
    cfg[:, 0] = np.float32(np.asarray(threshold_scale).reshape(-1)[0])
    cfg[:, 1] = np.float32(np.asarray(adaptation_bias).reshape(-1)[0])
    eye99_host = np.ascontiguousarray(
        np.eye(G, dtype=np.float32) * np.float32(EMA_DECAY))
    eyea_host = np.ascontiguousarray(
        np.eye(G, dtype=np.float32) * np.float32(ALPHA))
    eye1_host = np.ascontiguousarray(np.eye(G, dtype=np.float32))
    in_maps = []
    for j in range(NCORES):
        XTj = np.roll(XT, -j * SHARD, axis=2) if j else XT
        in_maps.append({
            "xt": np.ascontiguousarray(XTj.reshape(T * G, W)),
            "cfg": cfg,
            "eye": eye99_host,
            "eyea": eyea_host,
            "eye1": eye1_host,
        })
    return in_maps


def _postprocess(results, threshold_scale, adaptation_bias, x_sums):
    spikes = np.zeros((G, W, T), np.float32)
    for j in range(NCORES):
        blk = results[j]["spk"].reshape(G, SHARD, T)
        spikes[:, j * SHARD:(j + 1) * SHARD, :] = blk
    spikes = spikes.reshape(B, F, T)

    aux = results[0]["aux"].astype(np.float64)
    sums = aux[:, 0:T].sum(axis=0)
    mlast = aux[:, 2 * T].sum()
    asums = aux[:, 2 * T + 1:3 * T + 2].sum(axis=0)

    # syn column sums via host linear recurrence over x column sums
    ssyn = np.zeros(T)
    acc = 0.0
    for t in range(T):
        acc = BETA * acc + x_sums[t]
        ssyn[t] = acc

    base = np.float32(
        np.float32(np.asarray(threshold_scale).reshape(-1)[0])
        + np.float32(np.asarray(adaptation_bias).reshape(-1)[0]))
    Ef = np.float32(0.0)
    Cv = base
    mem_trace = np.zeros(T, np.float32)
    th_trace = np.zeros(T, np.float32)
    for t in range(T):
        th_trace[t] = np.float32(np.float32(K) * np.float32(asums[t] / N) + Cv)
        Ef = np.float32(np.float32(EMA_DECAY) * Ef
                        + np.float32(np.float32(C1) * np.float32(sums[t])))
        r1v = max(Ef - np.float32(HOMEO_RATE), np.float32(0.0))
        Cv = np.float32(np.float32(ADAPT_STRENGTH) * r1v + base)
        if t < T - 1:
            smp = asums[t + 1] - GAMMA * asums[t]   # sum m_{t+1}
            mem_trace[t] = np.float32((smp - ssyn[t + 1]) / ALPHA / N)
        else:
            mem_trace[t] = np.float32(mlast / N)
    ema = Ef
    adapt_mean = np.float32((1.0 - GAMMA) * asums[T - 1] / N)
    return spikes, mem_trace, th_trace, ema, adapt_mean


def kernel(input_current, threshold_scale, adaptation_bias):
    from concourse.bass_utils import run_bass_kernel_spmd

    if "nc" not in _CACHE:
        _CACHE["nc"] = _build()
    in_maps = _prep_inputs(input_current, threshold_scale, adaptation_bias)
    x_sums = np.asarray(input_current, np.float64).reshape(N, T).sum(axis=0)
    res = run_bass_kernel_spmd(_CACHE["nc"], in_maps,
                               core_ids=list(range(NCORES)))
    return _postprocess(res.results, threshold_scale, adaptation_bias, x_sums)


# revision 25
# speedup vs baseline: 1.0694x; 1.0694x over previous
"""AdaptiveThresholdLIFNeuron Trainium2 kernel (8 NeuronCores, SPMD).

The per-step global spike-rate EMA couples every element each timestep; on
this toolchain any cross-core exchange costs >=5us/step (collective floor;
the SWDGE remote-DMA ucode is absent from the runtime image), i.e. >=1.3ms
for T=256 just in communication. So the elementwise recurrence is
REPLICATED: every core runs the identical full-width [128 x 1024] chain
(bit-identical EMA evolution, zero cross-core traffic) and each core writes
only its own 1/8 of the spike output.

Host prep is layout-only: x [B,F,T] -> time-major XT[t][g][w] so the device
streams 512KB/step contiguously; each core's input is column-rotated so its
output shard sits at columns [0:128).

Engine schedule per step t (f32, W=1024 columns; DVE owns cols [0:DCOL),
Pool owns [DCOL:W) for the four split ops):
  Act:  TH = K*A + C ; THa = alpha*TH (pool half) ; u8 out of SPK shard ;
        r1 = relu(E - 0.01) ; C = 0.1*r1 + base ; SYb = beta*SY (pool
        half) ; Ma = alpha*M (pool half)
  DVE:  SPK = (M >= TH) [accum -> S_t] ; R.d = (alpha*SPK)*TH ;
        E = 0.99E + Spsum ; M.d = M0 - R ; A = gamma*A + M [accum] ;
        SY.d = beta*SY + X[t+2] ; M0.d = alpha*M + SY
  Pool: R.p = SPK*THa ; M.p = M0 - R ; SY.p = SYb + X[t+2] ;
        M0.p = Ma + SY
  PE:   Spsum = (c*ones) @ sums[:, t]     (partition reduce + broadcast)

M0 is software-pipelined one step ahead so the Act TH op overlaps the
SY/M0 updates. mem/th traces, ema and adapt-mean are finished on the host
from exact per-step sums.
"""

import numpy as np

B, F, T = 32, 4096, 256
N = B * F
G = 128
W = N // G                    # 1024
NCORES = 8
SHARD = W // NCORES           # 128
DCOL = 880                    # DVE columns of split ops; Pool gets W-DCOL

THRESHOLD_BASE = 1.0
DT = 0.001
ALPHA = float(np.exp(-DT / 0.02))
BETA = float(np.exp(-DT / 0.005))
GAMMA = float(np.exp(-DT / 0.1))
EMA_DECAY = 0.99
ADAPT_STRENGTH = 0.1
HOMEO_RATE = 0.01
K = float(np.float32(0.1) * (np.float32(1.0) - np.float32(GAMMA)))
C1 = float(np.float32(0.01) / np.float32(N))

BK = 8
NBLK = T // BK

_CACHE = {}

# ------------------------------------------------------------ V ledger
# V prologue (15 ops): 1 aux-memset, 2 SY-ms, 3 M-ms, 4 A-ms, 5 E-ms,
# 6 R-ms, 7 cones-ms, 8 negHR-ms, 9 base, 10 Ccopy, 11 SY0 (syn_0),
# 12 Mp (m_0), 13 A0 (asums[0]), 14 SYup (syn_1), 15 M0init.
# Per step t<=253 (6): SPK, R.d, MFIX.d, A, SY.d, M0.d
# t=254 (4): SPK, R.d, MFIX.d, A ; t=255 (3): SPK, Rfin, MPL
PRE = 15


def _vbase(t):
    return PRE + 6 * t


def _idx_spk(t):
    if t == T - 1:
        return _vbase(T - 2) + 4 + 1
    return _vbase(t) + 1


def _idx_A(t):
    # V op producing "A entering step t"
    return 13 if t == 0 else _vbase(t - 1) + 4


def _idx_syd(t):
    # V SY.d update issued during step t (produces syn_{t+2})
    return _vbase(t) + 5


def _final_dve():
    return _vbase(T - 2) + 4 + 3


# ---------------------------------------------------------- Pool ledger
# per step t<=253 (4): R.p, MFIX.p, SY.p, M0.p ; t=254 (2): R.p, MFIX.p
def _p_rp(t):
    return 4 * t + 1


def _p_mfix(t):
    return 4 * t + 2


def _p_sy(t):
    return 4 * t + 3


def _final_pool():
    return 4 * (T - 2) + 2


# ----------------------------------------------------------- Act ledger
# per step t<=253 (8): TH, THa, u8, Ecopy, r1, C, SYb, Ma
# t=254 (6): TH, THa, u8, Ecopy, r1, C ; t=255 (2): TH, u8
def _a_th(t):
    if t == T - 1:
        return 8 * (T - 2) + 6 + 1
    return 8 * t + 1


def _a_tha(t):
    return 8 * t + 2


def _a_r1(t):
    return 8 * t + 5


def _a_syb(t):
    return 8 * t + 7


def _a_ma(t):
    return 8 * t + 8


def _final_act():
    return _a_th(T - 1) + 1


def _build():
    import concourse.bass as bass
    import concourse.bacc as bacc
    import concourse.mybir as mybir

    f32 = mybir.dt.float32
    u8 = mybir.dt.uint8
    AOP = mybir.AluOpType
    ACT = mybir.ActivationFunctionType

    nc = bacc.Bacc(None, target_bir_lowering=False, debug=False)

    xt = nc.declare_dram_parameter("xt", [T * G, W], f32, isOutput=False)
    cfg = nc.declare_dram_parameter("cfg", [G, 2], f32, isOutput=False)
    eye = nc.declare_dram_parameter("eye", [G, G], f32, isOutput=False)
    spk_out = nc.declare_dram_parameter("spk", [G, SHARD * T], u8, isOutput=True)
    AUXW = 4 * T + 2
    aux_out = nc.declare_dram_parameter("aux", [G, AUXW], f32, isOutput=True)

    PCOL = W - DCOL

    from contextlib import ExitStack
    es = ExitStack()
    with es:
        sb = lambda name, shape, dt_: es.enter_context(
            nc.sbuf_tensor(name, shape, dt_))
        xbuf = sb("xbuf", [G, 2 * BK * W], f32)
        SY = sb("SY", [G, W], f32)
        M = sb("M", [G, W], f32)
        A_ = sb("A_", [G, W], f32)
        TH = sb("TH", [G, W], f32)
        SPK = sb("SPK", [G, W], f32)
        R = sb("R", [G, W], f32)
        M0 = sb("M0", [G, W], f32)
        THA = sb("THA", [G, PCOL], f32)
        SYB = sb("SYB", [G, PCOL], f32)
        MA = sb("MA", [G, PCOL], f32)
        u8acc = sb("u8acc", [G, SHARD * T], u8)
        aux = sb("auxb", [G, AUXW], f32)
        E = sb("E", [G, 1], f32)
        baset = sb("base", [G, 1], f32)
        C = sb("C", [G, 1], f32)
        r1 = sb("r1", [G, 1], f32)
        negHR = sb("negHR", [G, 1], f32)
        cfgb = sb("cfgb", [G, 2], f32)
        cones = sb("cones", [G, G], f32)
        eye99 = sb("eye99", [G, G], f32)
        Spsum = es.enter_context(nc.psum_tensor("Spsum", [G, 1], f32))
        dma_in = es.enter_context(nc.semaphore("dma_in"))
        cfg_sem = es.enter_context(nc.semaphore("cfg_sem"))
        dsem = es.enter_context(nc.semaphore("dsem"))
        asem = es.enter_context(nc.semaphore("asem"))
        poolc = es.enter_context(nc.semaphore("poolc"))
        psem = es.enter_context(nc.semaphore("psem"))
        osem = es.enter_context(nc.semaphore("osem"))
        block = es.enter_context(nc.Block())

        def xslice(t):
            s = (t % (2 * BK)) * W
            return xbuf[:, s:s + W]

        def xslice_d(t):
            s = (t % (2 * BK)) * W
            return xbuf[:, s:s + DCOL]

        def xslice_p(t):
            s = (t % (2 * BK)) * W + DCOL
            return xbuf[:, s:s + PCOL]

        dd = slice(0, DCOL)
        pp = slice(DCOL, W)

        @block.sync
        def _(sync):
            sync.dma_start(out=cfgb[:, :], in_=cfg[:, :]).then_inc(cfg_sem, 16)
            sync.dma_start(out=eye99[:, :], in_=eye[:, :]).then_inc(cfg_sem, 16)
            for b in range(NBLK):
                if b >= 2:
                    tcons = (b - 1) * BK - 3
                    sync.wait_ge(dsem, _idx_syd(tcons))
                    sync.wait_ge(poolc, _p_sy(tcons))
                sync.dma_start(
                    out=xbuf[:, ((b % 2) * BK * W):((b % 2) * BK * W + BK * W)]
                    .rearrange("g (t w) -> g t w", t=BK),
                    in_=xt[b * BK * G:(b + 1) * BK * G, :].rearrange(
                        "(t g) w -> g t w", g=G
                    ),
                ).then_inc(dma_in, 16)

        @block.vector
        def _(vector):
            cnt = [0]

            def V(ins):
                cnt[0] += 1
                return ins.then_inc(dsem, 1)

            def vw(sem, val):
                if val > 0:
                    vector.wait_ge(sem, val)

            def selfw():
                vw(dsem, cnt[0])

            # prologue 1..14
            V(vector.memset(aux[:, :], 0.0))
            V(vector.memset(SY[:, :], 0.0))
            V(vector.memset(M[:, :], 0.0))
            V(vector.memset(A_[:, :], 0.0))
            V(vector.memset(E[:, :], 0.0))
            V(vector.memset(R[:, :], 0.0))
            V(vector.memset(cones[:, :], C1))
            V(vector.memset(negHR[:, :], -float(HOMEO_RATE)))
            vw(cfg_sem, 32)
            V(vector.tensor_tensor(out=baset[:, :], in0=cfgb[:, 0:1],
                                   in1=cfgb[:, 1:2], op=AOP.add))
            selfw()
            V(vector.tensor_copy(out=C[:, :], in_=baset[:, :]))
            vw(dma_in, 16)
            selfw()
            V(vector.scalar_tensor_tensor(
                out=SY[:, :], in0=SY[:, :], scalar=BETA, in1=xslice(0),
                op0=AOP.mult, op1=AOP.add))
            selfw()
            V(vector.scalar_tensor_tensor(
                out=M[:, :], in0=M[:, :], scalar=ALPHA, in1=SY[:, :],
                op0=AOP.mult, op1=AOP.add))
            selfw()
            V(vector.scalar_tensor_tensor(
                out=A_[:, :], in0=A_[:, :], scalar=GAMMA, in1=M[:, :],
                op0=AOP.mult, op1=AOP.add,
                accum_out=aux[:, 2 * T + 1:2 * T + 2]))
            selfw()
            V(vector.scalar_tensor_tensor(
                out=SY[:, :], in0=SY[:, :], scalar=BETA, in1=xslice(1),
                op0=AOP.mult, op1=AOP.add))
            selfw()
            V(vector.scalar_tensor_tensor(
                out=M0[:, :], in0=M[:, :], scalar=ALPHA, in1=SY[:, :],
                op0=AOP.mult, op1=AOP.add))

            for t in range(T):
                # SPK (full width, accum -> S_t)
                vw(asem, _a_th(t))
                if t >= 1:
                    vw(poolc, _p_mfix(t - 1))
                selfw()
                V(vector.scalar_tensor_tensor(
                    out=SPK[:, :], in0=M[:, :], scalar=0.0, in1=TH[:, :],
                    op0=AOP.bypass, op1=AOP.is_ge,
                    accum_out=aux[:, t:t + 1]))
                if t == T - 1:
                    selfw()
                    V(vector.scalar_tensor_tensor(
                        out=R[:, :], in0=SPK[:, :], scalar=1.0, in1=TH[:, :],
                        op0=AOP.mult, op1=AOP.mult))
                    selfw()
                    V(vector.scalar_tensor_tensor(
                        out=M0[:, :], in0=R[:, :], scalar=-1.0, in1=M[:, :],
                        op0=AOP.mult, op1=AOP.add,
                        accum_out=aux[:, 2 * T:2 * T + 1]))
                    break
                # R.d
                selfw()
                V(vector.scalar_tensor_tensor(
                    out=R[:, dd], in0=SPK[:, dd], scalar=ALPHA, in1=TH[:, dd],
                    op0=AOP.mult, op1=AOP.mult))
                # MFIX.d
                selfw()
                V(vector.scalar_tensor_tensor(
                    out=M[:, dd], in0=R[:, dd], scalar=-1.0, in1=M0[:, dd],
                    op0=AOP.mult, op1=AOP.add))
                # A (full width; needs pool MFIX half too)
                vw(poolc, _p_mfix(t))
                selfw()
                V(vector.scalar_tensor_tensor(
                    out=A_[:, :], in0=A_[:, :], scalar=GAMMA, in1=M[:, :],
                    op0=AOP.mult, op1=AOP.add,
                    accum_out=aux[:, 2 * T + 1 + t + 1:2 * T + 2 + t + 1]))
                if t <= T - 3:
                    # SY.d (syn_{t+2})
                    vw(dma_in, 16 * ((t + 2) // BK + 1))
                    selfw()
                    V(vector.scalar_tensor_tensor(
                        out=SY[:, dd], in0=SY[:, dd], scalar=BETA,
                        in1=xslice_d(t + 2), op0=AOP.mult, op1=AOP.add))
                    # M0.d
                    selfw()
                    V(vector.scalar_tensor_tensor(
                        out=M0[:, dd], in0=M[:, dd], scalar=ALPHA,
                        in1=SY[:, dd], op0=AOP.mult, op1=AOP.add))

        @block.scalar
        def _(scalar):
            cnt = [0]

            def S(ins):
                cnt[0] += 1
                return ins.then_inc(asem, 1)

            def sw(sem, val):
                if val > 0:
                    scalar.wait_ge(sem, val)

            for t in range(T):
                # TH
                sw(dsem, max(_idx_A(t), 10))
                if t >= 1:
                    sw(poolc, _p_rp(t - 1))
                sw(asem, cnt[0])
                S(scalar.activation(
                    out=TH[:, :], in_=A_[:, :], func=ACT.Identity,
                    scale=K, bias=C[:, :]))
                if t == T - 1:
                    sw(dsem, _idx_spk(t))
                    sw(asem, cnt[0])
                    S(scalar.activation(
                        out=u8acc[:, t:t + (SHARD - 1) * T + 1:T],
                        in_=SPK[:, 0:SHARD], func=ACT.Copy))
                    break
                # THa (pool half)
                sw(asem, cnt[0])
                S(scalar.activation(
                    out=THA[:, :], in_=TH[:, pp], func=ACT.Identity,
                    scale=ALPHA))
                # u8
                sw(dsem, _idx_spk(t))
                sw(asem, cnt[0])
                S(scalar.activation(
                    out=u8acc[:, t:t + (SHARD - 1) * T + 1:T],
                    in_=SPK[:, 0:SHARD], func=ACT.Copy))
                # Ecopy: E <- Epsum (E' from PE)
                sw(psem, 2 * (t + 1))
                sw(asem, cnt[0])
                S(scalar.activation(
                    out=E[:, :], in_=Spsum[:, :], func=ACT.Copy))
                # r1 = relu(E' - 0.01)  (read from psum)
                sw(asem, cnt[0])
                S(scalar.activation(
                    out=r1[:, :], in_=Spsum[:, :], func=ACT.Relu,
                    scale=1.0, bias=negHR[:, :]))
                # C = 0.1*r1 + base
                sw(asem, cnt[0])
                S(scalar.activation(
                    out=C[:, :], in_=r1[:, :], func=ACT.Identity,
                    scale=float(ADAPT_STRENGTH), bias=baset[:, :]))
                if t <= T - 3:
                    # SYb (pool half)
                    if t >= 1:
                        sw(poolc, _p_sy(t - 1))
                    else:
                        sw(dsem, 14)
                    sw(asem, cnt[0])
                    S(scalar.activation(
                        out=SYB[:, :], in_=SY[:, pp], func=ACT.Identity,
                        scale=BETA))
                    # Ma (pool half)
                    sw(poolc, _p_mfix(t))
                    sw(asem, cnt[0])
                    S(scalar.activation(
                        out=MA[:, :], in_=M[:, pp], func=ACT.Identity,
                        scale=ALPHA))

        @block.tensor
        def _(tensor):
            tensor.wait_ge(cfg_sem, 32)
            tensor.wait_ge(dsem, 7)   # E memset (5), cones memset (7)
            for t in range(T):
                tensor.wait_ge(dsem, _idx_spk(t))
                if t > 0:
                    tensor.wait_ge(asem, _a_r1(t - 1))
                tensor.matmul(
                    Spsum[:, :], eye99[:, :], E[:, :],
                    start=True, stop=False,
                ).then_inc(psem, 1)
                tensor.matmul(
                    Spsum[:, :], cones[:, :], aux[:, t:t + 1],
                    start=False, stop=True,
                ).then_inc(psem, 1)

        @block.gpsimd
        def _(pool):
            cnt = [0]

            def P(ins):
                cnt[0] += 1
                return ins.then_inc(poolc, 1)

            def pw(sem, val):
                if val > 0:
                    pool.wait_ge(sem, val)

            for t in range(T - 1):
                # R.p = SPK * THA
                pw(dsem, _idx_spk(t))
                pw(asem, _a_tha(t))
                pw(poolc, cnt[0])
                P(pool.tensor_tensor(out=R[:, pp], in0=SPK[:, pp],
                                     in1=THA[:, :], op=AOP.mult))
                # MFIX.p = M0 - R  (t=0: M0 pool half from V prologue)
                pw(poolc, cnt[0])
                if t == 0:
                    pw(dsem, PRE)
                P(pool.tensor_tensor(out=M[:, pp], in0=M0[:, pp],
                                     in1=R[:, pp], op=AOP.subtract))
                if t <= T - 3:
                    # SY.p = SYB + X[t+2]
                    pw(asem, _a_syb(t))
                    pw(dma_in, 16 * ((t + 2) // BK + 1))
                    pw(poolc, cnt[0])
                    P(pool.tensor_tensor(out=SY[:, pp], in0=SYB[:, :],
                                         in1=xslice_p(t + 2), op=AOP.add))
                    # M0.p = MA + SY
                    pw(asem, _a_ma(t))
                    pw(poolc, cnt[0])
                    P(pool.tensor_tensor(out=M0[:, pp], in0=MA[:, :],
                                         in1=SY[:, pp], op=AOP.add))

            pool.wait_ge(dsem, _final_dve())
            pool.wait_ge(asem, _final_act())
            pool.wait_ge(poolc, _final_pool())
            pool.dma_start(out=spk_out[:, :], in_=u8acc[:, :]).then_inc(osem, 16)
            pool.dma_start(out=aux_out[:, :], in_=aux[:, :]).then_inc(osem, 16)
            pool.wait_ge(osem, 32)

    nc.finalize()
    return nc


def _prep_inputs(input_current, threshold_scale, adaptation_bias):
    x = np.ascontiguousarray(
        np.asarray(input_current, np.float32).reshape(N, T))
    XT = np.ascontiguousarray(x.reshape(G, W, T).transpose(2, 0, 1))
    cfg = np.zeros((G, 2), np.float32)
    cfg[:, 0] = np.float32(np.asarray(threshold_scale).reshape(-1)[0])
    cfg[:, 1] = np.float32(np.asarray(adaptation_bias).reshape(-1)[0])
    eye99_host = np.ascontiguousarray(
        np.eye(G, dtype=np.float32) * np.float32(EMA_DECAY))
    in_maps = []
    for j in range(NCORES):
        XTj = np.roll(XT, -j * SHARD, axis=2) if j else XT
        in_maps.append({
            "xt": np.ascontiguousarray(XTj.reshape(T * G, W)),
            "cfg": cfg,
            "eye": eye99_host,
        })
    return in_maps


def _postprocess(results, threshold_scale, adaptation_bias, x_sums):
    spikes = np.zeros((G, W, T), np.float32)
    for j in range(NCORES):
        blk = results[j]["spk"].reshape(G, SHARD, T)
        spikes[:, j * SHARD:(j + 1) * SHARD, :] = blk
    spikes = spikes.reshape(B, F, T)

    aux = results[0]["aux"].astype(np.float64)
    sums = aux[:, 0:T].sum(axis=0)
    mlast = aux[:, 2 * T].sum()
    asums = aux[:, 2 * T + 1:3 * T + 2].sum(axis=0)

    # syn column sums via host linear recurrence over x column sums
    ssyn = np.zeros(T)
    acc = 0.0
    for t in range(T):
        acc = BETA * acc + x_sums[t]
        ssyn[t] = acc

    base = np.float32(
        np.float32(np.asarray(threshold_scale).reshape(-1)[0])
        + np.float32(np.asarray(adaptation_bias).reshape(-1)[0]))
    Ef = np.float32(0.0)
    Cv = base
    mem_trace = np.zeros(T, np.float32)
    th_trace = np.zeros(T, np.float32)
    for t in range(T):
        th_trace[t] = np.float32(np.float32(K) * np.float32(asums[t] / N) + Cv)
        Ef = np.float32(np.float32(EMA_DECAY) * Ef
                        + np.float32(np.float32(C1) * np.float32(sums[t])))
        r1v = max(Ef - np.float32(HOMEO_RATE), np.float32(0.0))
        Cv = np.float32(np.float32(ADAPT_STRENGTH) * r1v + base)
        if t < T - 1:
            smp = asums[t + 1] - GAMMA * asums[t]   # sum m_{t+1}
            mem_trace[t] = np.float32((smp - ssyn[t + 1]) / ALPHA / N)
        else:
            mem_trace[t] = np.float32(mlast / N)
    ema = Ef
    adapt_mean = np.float32((1.0 - GAMMA) * asums[T - 1] / N)
    return spikes, mem_trace, th_trace, ema, adapt_mean


def kernel(input_current, threshold_scale, adaptation_bias):
    from concourse.bass_utils import run_bass_kernel_spmd

    if "nc" not in _CACHE:
        _CACHE["nc"] = _build()
    in_maps = _prep_inputs(input_current, threshold_scale, adaptation_bias)
    x_sums = np.asarray(input_current, np.float64).reshape(N, T).sum(axis=0)
    res = run_bass_kernel_spmd(_CACHE["nc"], in_maps,
                               core_ids=list(range(NCORES)))
    return _postprocess(res.results, threshold_scale, adaptation_bias, x_sums)


# revision 27
# speedup vs baseline: 1.2708x; 1.1884x over previous
"""AdaptiveThresholdLIFNeuron Trainium2 kernel (8 NeuronCores, SPMD).

The per-step global spike-rate EMA couples every element each timestep; on
this toolchain any cross-core exchange costs >=5us/step (collective floor;
the SWDGE remote-DMA ucode is absent from the runtime image), i.e. >=1.3ms
for T=256 just in communication. So the elementwise recurrence is
REPLICATED: every core runs the identical full-width [128 x 1024] chain
(bit-identical EMA evolution, zero cross-core traffic) and each core writes
only its own 1/8 of the spike output.

Host prep is layout-only: x [B,F,T] -> time-major XT[t][g][w] so the device
streams 512KB/step contiguously; each core's input is column-rotated so its
output shard sits at columns [0:128).

Engine schedule per step t (f32, W=1024 columns; DVE owns cols [0:DCOL),
Pool owns [DCOL:W) for the four split ops):
  Act:  TH = K*A + C ; THa = alpha*TH (pool half) ; u8 out of SPK shard ;
        r1 = relu(E - 0.01) ; C = 0.1*r1 + base ; SYb = beta*SY (pool
        half) ; Ma = alpha*M (pool half)
  DVE:  SPK = (M >= TH) [accum -> S_t] ; R.d = (alpha*SPK)*TH ;
        E = 0.99E + Spsum ; M.d = M0 - R ; A = gamma*A + M [accum] ;
        SY.d = beta*SY + X[t+2] ; M0.d = alpha*M + SY
  Pool: R.p = SPK*THa ; M.p = M0 - R ; SY.p = SYb + X[t+2] ;
        M0.p = Ma + SY
  PE:   Spsum = (c*ones) @ sums[:, t]     (partition reduce + broadcast)

M0 is software-pipelined one step ahead so the Act TH op overlaps the
SY/M0 updates. mem/th traces, ema and adapt-mean are finished on the host
from exact per-step sums.
"""

import numpy as np

B, F, T = 32, 4096, 256
N = B * F
G = 128
W = N // G                    # 1024
NCORES = 8
SHARD = W // NCORES           # 128
DCOL = 864                    # DVE columns of split ops; Pool gets W-DCOL

THRESHOLD_BASE = 1.0
DT = 0.001
ALPHA = float(np.exp(-DT / 0.02))
BETA = float(np.exp(-DT / 0.005))
GAMMA = float(np.exp(-DT / 0.1))
EMA_DECAY = 0.99
ADAPT_STRENGTH = 0.1
HOMEO_RATE = 0.01
K = float(np.float32(0.1) * (np.float32(1.0) - np.float32(GAMMA)))
C1 = float(np.float32(0.01) / np.float32(N))

BK = 8
NBLK = T // BK

_CACHE = {}

# ------------------------------------------------------------ V ledger
# V prologue (15 ops): 1 aux-memset, 2 SY-ms, 3 M-ms, 4 A-ms, 5 E-ms,
# 6 R-ms, 7 cones-ms, 8 negHR-ms, 9 base, 10 Ccopy, 11 SY0 (syn_0),
# 12 Mp (m_0), 13 A0 (asums[0]), 14 SYup (syn_1), 15 M0init.
# Per step t<=253 (6): SPK, R.d, MFIX.d, A, SY.d, M0.d
# t=254 (4): SPK, R.d, MFIX.d, A ; t=255 (3): SPK, Rfin, MPL
PRE = 15


def _vbase(t):
    return PRE + 6 * t


def _idx_spk(t):
    if t == T - 1:
        return _vbase(T - 2) + 4 + 1
    return _vbase(t) + 1


def _idx_A(t):
    # V op producing "A entering step t"
    return 13 if t == 0 else _vbase(t - 1) + 4


def _idx_syd(t):
    # V SY.d update issued during step t (produces syn_{t+2})
    return _vbase(t) + 5


def _final_dve():
    return _vbase(T - 2) + 4 + 3


# ---------------------------------------------------------- Pool ledger
# per step t<=253 (4): R.p, MFIX.p, SY.p, M0.p ; t=254 (2): R.p, MFIX.p
def _p_rp(t):
    return 4 * t + 1


def _p_mfix(t):
    return 4 * t + 2


def _p_sy(t):
    return 4 * t + 3


def _final_pool():
    return 4 * (T - 2) + 2


# ----------------------------------------------------------- Act ledger
# per step t<=253 (8): TH, THa, u8, Ecopy, r1, C, SYb, Ma
# t=254 (6): TH, THa, u8, Ecopy, r1, C ; t=255 (2): TH, u8
def _a_th(t):
    if t == T - 1:
        return 8 * (T - 2) + 6 + 1
    return 8 * t + 1


def _a_tha(t):
    return 8 * t + 2


def _a_r1(t):
    return 8 * t + 5


def _a_syb(t):
    return 8 * t + 7


def _a_ma(t):
    return 8 * t + 8


def _final_act():
    return _a_th(T - 1) + 1


def _build():
    import concourse.bass as bass
    import concourse.bacc as bacc
    import concourse.mybir as mybir

    f32 = mybir.dt.float32
    u8 = mybir.dt.uint8
    AOP = mybir.AluOpType
    ACT = mybir.ActivationFunctionType

    nc = bacc.Bacc(None, target_bir_lowering=False, debug=False)

    xt = nc.declare_dram_parameter("xt", [T * G, W], f32, isOutput=False)
    cfg = nc.declare_dram_parameter("cfg", [G, 2], f32, isOutput=False)
    eye = nc.declare_dram_parameter("eye", [G, G], f32, isOutput=False)
    spk_out = nc.declare_dram_parameter("spk", [G, SHARD * T], u8, isOutput=True)
    AUXW = 4 * T + 2
    aux_out = nc.declare_dram_parameter("aux", [G, AUXW], f32, isOutput=True)

    PCOL = W - DCOL

    from contextlib import ExitStack
    es = ExitStack()
    with es:
        sb = lambda name, shape, dt_: es.enter_context(
            nc.sbuf_tensor(name, shape, dt_))
        xbuf = sb("xbuf", [G, 2 * BK * W], f32)
        SY = sb("SY", [G, W], f32)
        M = sb("M", [G, W], f32)
        A_ = sb("A_", [G, W], f32)
        TH = sb("TH", [G, W], f32)
        SPK = sb("SPK", [G, W], f32)
        R = sb("R", [G, W], f32)
        M0 = sb("M0", [G, W], f32)
        THA = sb("THA", [G, PCOL], f32)
        SYB = sb("SYB", [G, PCOL], f32)
        MA = sb("MA", [G, PCOL], f32)
        u8acc = sb("u8acc", [G, SHARD * T], u8)
        aux = sb("auxb", [G, AUXW], f32)
        E = sb("E", [G, 1], f32)
        baset = sb("base", [G, 1], f32)
        C = sb("C", [G, 1], f32)
        r1 = sb("r1", [G, 1], f32)
        negHR = sb("negHR", [G, 1], f32)
        cfgb = sb("cfgb", [G, 2], f32)
        cones = sb("cones", [G, G], f32)
        eye99 = sb("eye99", [G, G], f32)
        Spsum = es.enter_context(nc.psum_tensor("Spsum", [G, 1], f32))
        dma_in = es.enter_context(nc.semaphore("dma_in"))
        cfg_sem = es.enter_context(nc.semaphore("cfg_sem"))
        dsem = es.enter_context(nc.semaphore("dsem"))
        asem = es.enter_context(nc.semaphore("asem"))
        poolc = es.enter_context(nc.semaphore("poolc"))
        psem = es.enter_context(nc.semaphore("psem"))
        osem = es.enter_context(nc.semaphore("osem"))
        block = es.enter_context(nc.Block())

        def xslice(t):
            s = (t % (2 * BK)) * W
            return xbuf[:, s:s + W]

        def xslice_d(t):
            s = (t % (2 * BK)) * W
            return xbuf[:, s:s + DCOL]

        def xslice_p(t):
            s = (t % (2 * BK)) * W + DCOL
            return xbuf[:, s:s + PCOL]

        dd = slice(0, DCOL)
        pp = slice(DCOL, W)

        @block.sync
        def _(sync):
            sync.dma_start(out=cfgb[:, :], in_=cfg[:, :]).then_inc(cfg_sem, 16)
            sync.dma_start(out=eye99[:, :], in_=eye[:, :]).then_inc(cfg_sem, 16)
            for b in range(NBLK):
                if b >= 2:
                    tcons = (b - 1) * BK - 3
                    sync.wait_ge(dsem, _idx_syd(tcons))
                    sync.wait_ge(poolc, _p_sy(tcons))
                sync.dma_start(
                    out=xbuf[:, ((b % 2) * BK * W):((b % 2) * BK * W + BK * W)]
                    .rearrange("g (t w) -> g t w", t=BK),
                    in_=xt[b * BK * G:(b + 1) * BK * G, :].rearrange(
                        "(t g) w -> g t w", g=G
                    ),
                ).then_inc(dma_in, 16)

        @block.vector
        def _(vector):
            cnt = [0]

            def V(ins):
                cnt[0] += 1
                return ins.then_inc(dsem, 1)

            def vw(sem, val):
                if val > 0:
                    vector.wait_ge(sem, val)

            def selfw():
                vw(dsem, cnt[0])

            # prologue 1..14
            V(vector.memset(aux[:, :], 0.0))
            V(vector.memset(SY[:, :], 0.0))
            V(vector.memset(M[:, :], 0.0))
            V(vector.memset(A_[:, :], 0.0))
            V(vector.memset(E[:, :], 0.0))
            V(vector.memset(R[:, :], 0.0))
            V(vector.memset(cones[:, :], C1))
            V(vector.memset(negHR[:, :], -float(HOMEO_RATE)))
            vw(cfg_sem, 32)
            V(vector.tensor_tensor(out=baset[:, :], in0=cfgb[:, 0:1],
                                   in1=cfgb[:, 1:2], op=AOP.add))
            selfw()
            V(vector.tensor_copy(out=C[:, :], in_=baset[:, :]))
            vw(dma_in, 16)
            selfw()
            V(vector.scalar_tensor_tensor(
                out=SY[:, :], in0=SY[:, :], scalar=BETA, in1=xslice(0),
                op0=AOP.mult, op1=AOP.add))
            selfw()
            V(vector.scalar_tensor_tensor(
                out=M[:, :], in0=M[:, :], scalar=ALPHA, in1=SY[:, :],
                op0=AOP.mult, op1=AOP.add))
            selfw()
            V(vector.scalar_tensor_tensor(
                out=A_[:, :], in0=A_[:, :], scalar=GAMMA, in1=M[:, :],
                op0=AOP.mult, op1=AOP.add,
                accum_out=aux[:, 2 * T + 1:2 * T + 2]))
            selfw()
            V(vector.scalar_tensor_tensor(
                out=SY[:, :], in0=SY[:, :], scalar=BETA, in1=xslice(1),
                op0=AOP.mult, op1=AOP.add))
            selfw()
            V(vector.scalar_tensor_tensor(
                out=M0[:, :], in0=M[:, :], scalar=ALPHA, in1=SY[:, :],
                op0=AOP.mult, op1=AOP.add))

            for t in range(T):
                # SPK (full width, accum -> S_t)
                vw(asem, _a_th(t))
                if t >= 1:
                    vw(poolc, _p_mfix(t - 1))
                selfw()
                V(vector.scalar_tensor_tensor(
                    out=SPK[:, :], in0=M[:, :], scalar=0.0, in1=TH[:, :],
                    op0=AOP.bypass, op1=AOP.is_ge,
                    accum_out=aux[:, t:t + 1]))
                if t == T - 1:
                    selfw()
                    V(vector.scalar_tensor_tensor(
                        out=R[:, :], in0=SPK[:, :], scalar=1.0, in1=TH[:, :],
                        op0=AOP.mult, op1=AOP.mult))
                    selfw()
                    V(vector.scalar_tensor_tensor(
                        out=M0[:, :], in0=R[:, :], scalar=-1.0, in1=M[:, :],
                        op0=AOP.mult, op1=AOP.add,
                        accum_out=aux[:, 2 * T:2 * T + 1]))
                    break
                # R.d
                selfw()
                V(vector.scalar_tensor_tensor(
                    out=R[:, dd], in0=SPK[:, dd], scalar=ALPHA, in1=TH[:, dd],
                    op0=AOP.mult, op1=AOP.mult))
                # MFIX.d
                selfw()
                V(vector.scalar_tensor_tensor(
                    out=M[:, dd], in0=R[:, dd], scalar=-1.0, in1=M0[:, dd],
                    op0=AOP.mult, op1=AOP.add))
                # A (full width; needs pool MFIX half too)
                vw(poolc, _p_mfix(t))
                selfw()
                V(vector.scalar_tensor_tensor(
                    out=A_[:, :], in0=A_[:, :], scalar=GAMMA, in1=M[:, :],
                    op0=AOP.mult, op1=AOP.add,
                    accum_out=aux[:, 2 * T + 1 + t + 1:2 * T + 2 + t + 1]))
                if t <= T - 3:
                    # SY.d (syn_{t+2})
                    vw(dma_in, 16 * ((t + 2) // BK + 1))
                    selfw()
                    V(vector.scalar_tensor_tensor(
                        out=SY[:, dd], in0=SY[:, dd], scalar=BETA,
                        in1=xslice_d(t + 2), op0=AOP.mult, op1=AOP.add))
                    # M0.d
                    selfw()
                    V(vector.scalar_tensor_tensor(
                        out=M0[:, dd], in0=M[:, dd], scalar=ALPHA,
                        in1=SY[:, dd], op0=AOP.mult, op1=AOP.add))

        @block.scalar
        def _(scalar):
            cnt = [0]

            def S(ins):
                cnt[0] += 1
                return ins.then_inc(asem, 1)

            def sw(sem, val):
                if val > 0:
                    scalar.wait_ge(sem, val)

            for t in range(T):
                # SYb/Ma for step t-1, emitted here so the pool-gated Ma
                # wait no longer head-of-line blocks TH_t in the Act queue
                # (SPK_t already gates on pool MFIX.p_{t-1} regardless).
                if 1 <= t <= T - 2:
                    tp = t - 1
                    if tp >= 1:
                        sw(poolc, _p_sy(tp - 1))
                    else:
                        sw(dsem, 14)
                    sw(asem, cnt[0])
                    S(scalar.activation(
                        out=SYB[:, :], in_=SY[:, pp], func=ACT.Identity,
                        scale=BETA))
                    sw(poolc, _p_mfix(tp))
                    sw(asem, cnt[0])
                    S(scalar.activation(
                        out=MA[:, :], in_=M[:, pp], func=ACT.Identity,
                        scale=ALPHA))
                # TH
                sw(dsem, max(_idx_A(t), 10))
                if t >= 1:
                    sw(poolc, _p_rp(t - 1))
                sw(asem, cnt[0])
                S(scalar.activation(
                    out=TH[:, :], in_=A_[:, :], func=ACT.Identity,
                    scale=K, bias=C[:, :]))
                if t == T - 1:
                    sw(dsem, _idx_spk(t))
                    sw(asem, cnt[0])
                    S(scalar.activation(
                        out=u8acc[:, t:t + (SHARD - 1) * T + 1:T],
                        in_=SPK[:, 0:SHARD], func=ACT.Copy))
                    break
                # THa (pool half)
                sw(asem, cnt[0])
                S(scalar.activation(
                    out=THA[:, :], in_=TH[:, pp], func=ACT.Identity,
                    scale=ALPHA))
                # u8
                sw(dsem, _idx_spk(t))
                sw(asem, cnt[0])
                S(scalar.activation(
                    out=u8acc[:, t:t + (SHARD - 1) * T + 1:T],
                    in_=SPK[:, 0:SHARD], func=ACT.Copy))
                # Ecopy: E <- Epsum (E' from PE)
                sw(psem, 2 * (t + 1))
                sw(asem, cnt[0])
                S(scalar.activation(
                    out=E[:, :], in_=Spsum[:, :], func=ACT.Copy))
                # r1 = relu(E' - 0.01)  (read from psum)
                sw(asem, cnt[0])
                S(scalar.activation(
                    out=r1[:, :], in_=Spsum[:, :], func=ACT.Relu,
                    scale=1.0, bias=negHR[:, :]))
                # C = 0.1*r1 + base
                sw(asem, cnt[0])
                S(scalar.activation(
                    out=C[:, :], in_=r1[:, :], func=ACT.Identity,
                    scale=float(ADAPT_STRENGTH), bias=baset[:, :]))


        @block.tensor
        def _(tensor):
            tensor.wait_ge(cfg_sem, 32)
            tensor.wait_ge(dsem, 7)   # E memset (5), cones memset (7)
            for t in range(T):
                tensor.wait_ge(dsem, _idx_spk(t))
                if t > 0:
                    tensor.wait_ge(asem, _a_r1(t - 1))
                tensor.matmul(
                    Spsum[:, :], eye99[:, :], E[:, :],
                    start=True, stop=False,
                ).then_inc(psem, 1)
                tensor.matmul(
                    Spsum[:, :], cones[:, :], aux[:, t:t + 1],
                    start=False, stop=True,
                ).then_inc(psem, 1)

        @block.gpsimd
        def _(pool):
            cnt = [0]

            def P(ins):
                cnt[0] += 1
                return ins.then_inc(poolc, 1)

            def pw(sem, val):
                if val > 0:
                    pool.wait_ge(sem, val)

            for t in range(T - 1):
                # R.p = SPK * THA
                pw(dsem, _idx_spk(t))
                pw(asem, _a_tha(t))
                pw(poolc, cnt[0])
                P(pool.tensor_tensor(out=R[:, pp], in0=SPK[:, pp],
                                     in1=THA[:, :], op=AOP.mult))
                # MFIX.p = M0 - R  (t=0: M0 pool half from V prologue)
                pw(poolc, cnt[0])
                if t == 0:
                    pw(dsem, PRE)
                P(pool.tensor_tensor(out=M[:, pp], in0=M0[:, pp],
                                     in1=R[:, pp], op=AOP.subtract))
                if t <= T - 3:
                    # SY.p = SYB + X[t+2]
                    pw(asem, _a_syb(t))
                    pw(dma_in, 16 * ((t + 2) // BK + 1))
                    pw(poolc, cnt[0])
                    P(pool.tensor_tensor(out=SY[:, pp], in0=SYB[:, :],
                                         in1=xslice_p(t + 2), op=AOP.add))
                    # M0.p = MA + SY
                    pw(asem, _a_ma(t))
                    pw(poolc, cnt[0])
                    P(pool.tensor_tensor(out=M0[:, pp], in0=MA[:, :],
                                         in1=SY[:, pp], op=AOP.add))

            pool.wait_ge(dsem, _final_dve())
            pool.wait_ge(asem, _final_act())
            pool.wait_ge(poolc, _final_pool())
            pool.dma_start(out=spk_out[:, :], in_=u8acc[:, :]).then_inc(osem, 16)
            pool.dma_start(out=aux_out[:, :], in_=aux[:, :]).then_inc(osem, 16)
            pool.wait_ge(osem, 32)

    nc.finalize()
    return nc


def _prep_inputs(input_current, threshold_scale, adaptation_bias):
    x = np.ascontiguousarray(
        np.asarray(input_current, np.float32).reshape(N, T))
    XT = np.ascontiguousarray(x.reshape(G, W, T).transpose(2, 0, 1))
    cfg = np.zeros((G, 2), np.float32)
    cfg[:, 0] = np.float32(np.asarray(threshold_scale).reshape(-1)[0])
    cfg[:, 1] = np.float32(np.asarray(adaptation_bias).reshape(-1)[0])
    eye99_host = np.ascontiguousarray(
        np.eye(G, dtype=np.float32) * np.float32(EMA_DECAY))
    in_maps = []
    for j in range(NCORES):
        XTj = np.roll(XT, -j * SHARD, axis=2) if j else XT
        in_maps.append({
            "xt": np.ascontiguousarray(XTj.reshape(T * G, W)),
            "cfg": cfg,
            "eye": eye99_host,
        })
    return in_maps


def _postprocess(results, threshold_scale, adaptation_bias, x_sums):
    spikes = np.zeros((G, W, T), np.float32)
    for j in range(NCORES):
        blk = results[j]["spk"].reshape(G, SHARD, T)
        spikes[:, j * SHARD:(j + 1) * SHARD, :] = blk
    spikes = spikes.reshape(B, F, T)

    aux = results[0]["aux"].astype(np.float64)
    sums = aux[:, 0:T].sum(axis=0)
    mlast = aux[:, 2 * T].sum()
    asums = aux[:, 2 * T + 1:3 * T + 2].sum(axis=0)

    # syn column sums via host linear recurrence over x column sums
    ssyn = np.zeros(T)
    acc = 0.0
    for t in range(T):
        acc = BETA * acc + x_sums[t]
        ssyn[t] = acc

    base = np.float32(
        np.float32(np.asarray(threshold_scale).reshape(-1)[0])
        + np.float32(np.asarray(adaptation_bias).reshape(-1)[0]))
    Ef = np.float32(0.0)
    Cv = base
    mem_trace = np.zeros(T, np.float32)
    th_trace = np.zeros(T, np.float32)
    for t in range(T):
        th_trace[t] = np.float32(np.float32(K) * np.float32(asums[t] / N) + Cv)
        Ef = np.float32(np.float32(EMA_DECAY) * Ef
                        + np.float32(np.float32(C1) * np.float32(sums[t])))
        r1v = max(Ef - np.float32(HOMEO_RATE), np.float32(0.0))
        Cv = np.float32(np.float32(ADAPT_STRENGTH) * r1v + base)
        if t < T - 1:
            smp = asums[t + 1] - GAMMA * asums[t]   # sum m_{t+1}
            mem_trace[t] = np.float32((smp - ssyn[t + 1]) / ALPHA / N)
        else:
            mem_trace[t] = np.float32(mlast / N)
    ema = Ef
    adapt_mean = np.float32((1.0 - GAMMA) * asums[T - 1] / N)
    return spikes, mem_trace, th_trace, ema, adapt_mean


def kernel(input_current, threshold_scale, adaptation_bias):
    from concourse.bass_utils import run_bass_kernel_spmd

    if "nc" not in _CACHE:
        _CACHE["nc"] = _build()
    in_maps = _prep_inputs(input_current, threshold_scale, adaptation_bias)
    x_sums = np.asarray(input_current, np.float64).reshape(N, T).sum(axis=0)
    res = run_bass_kernel_spmd(_CACHE["nc"], in_maps,
                               core_ids=list(range(NCORES)))
    return _postprocess(res.results, threshold_scale, adaptation_bias, x_sums)
